# revision 5
# baseline (speedup 1.0000x reference)
"""MoE with shared expert — Trainium2 Bass kernel, 8-core expert-parallel.

Strategy (per sharding hint): tokens are dispatched by argmax expert on the
host at input-sharding time; the 8 cores each hold ONE expert's weights plus
the replicated shared-expert weights, and run the fused two-layer MLP
(selected expert + shared expert, combined in PSUM) over their token slice.
The tiny router (T x 512 x 4 matmul + softmax) and the final gather/scatter
run on the host as part of sharding/unsharding; the device does the ~69
GFLOP of expert/shared MLP work.

Device kernel, per core, for capacity C tokens (chunks of <=512):
  hT_e  = gelu(W1e^T @ X^T + b1e)            [H, C]   (streamed per 128-row tile)
  acc   = W2e^T @ hT_e                        [D, C]   (PSUM, 4 banks)
  hT_s  = gelu(sW1^T @ X^T + sb1) * sw        [H, C]   (sw = per-token scale)
  acc  += sW2^T @ hT_s + b2e x 1 + sb2 x sw   (rank-1 bias matmuls, K=1)
All matmuls run as float32r (TF32-like, full PE rate at N>=256).
"""

import numpy as np
from contextlib import ExitStack

D, H, E = 512, 2048, 4
NCORES = 8
LBW = 0.01
P = 128
KD, KH, ND = D // P, H // P, D // P

# token capacity per core and the matmul chunk sizes that tile it
CHUNKS_PRIMARY = (512, 384, 256)     # C = 1152, covers expert counts <= 2304
CHUNKS_FALLBACK = (512, 512, 512, 512)  # C = 2048, covers any distribution

_prog_cache = {}


def _build_program(chunks, repeat=1):
    """Build the per-core Bacc program.  repeat>1 wraps the whole body
    (including weight loads) in a For_i loop — used only for timing."""
    import concourse.tile as tile
    from concourse import bacc, mybir

    f32 = mybir.dt.float32
    f32r = mybir.dt.float32r
    Gelu = mybir.ActivationFunctionType.Gelu
    C = sum(chunks)

    nc = bacc.Bacc("TRN2", target_bir_lowering=False, debug=False)
    xt = nc.dram_tensor("xt", [D, C], f32r, kind="ExternalInput")
    w1 = nc.dram_tensor("w1", [D, H], f32r, kind="ExternalInput")
    w2 = nc.dram_tensor("w2", [H, D], f32r, kind="ExternalInput")
    v1 = nc.dram_tensor("v1", [D, H], f32r, kind="ExternalInput")
    v2 = nc.dram_tensor("v2", [H, D], f32r, kind="ExternalInput")
    b1 = nc.dram_tensor("b1", [P, KH], f32, kind="ExternalInput")
    c1 = nc.dram_tensor("c1", [P, KH], f32, kind="ExternalInput")
    b2 = nc.dram_tensor("b2", [1, D], f32r, kind="ExternalInput")
    c2 = nc.dram_tensor("c2", [1, D], f32r, kind="ExternalInput")
    sw = nc.dram_tensor("sw", [1, C], f32r, kind="ExternalInput")
    on = nc.dram_tensor("on", [1, 512], f32r, kind="ExternalInput")
    out = nc.dram_tensor("out", [D, C], f32, kind="ExternalOutput")

    import concourse.bass as bass

    with tile.TileContext(nc) as tc, ExitStack() as ctx:
        wp = ctx.enter_context(tc.tile_pool(name="wp", bufs=1))
        xp = ctx.enter_context(tc.tile_pool(name="xp", bufs=2))
        hp = ctx.enter_context(tc.tile_pool(name="hp", bufs=4))
        sp = ctx.enter_context(tc.tile_pool(name="sp", bufs=2))
        op = ctx.enter_context(tc.tile_pool(name="op", bufs=2))
        l1p = ctx.enter_context(tc.tile_pool(name="l1p", bufs=3, space="PSUM"))
        l2p = ctx.enter_context(tc.tile_pool(name="l2p", bufs=4, space="PSUM"))

        if repeat > 1:
            loop_cm = tc.For_i(0, repeat, 1,
                               hint_engines=(mybir.EngineType.PE,))
            loop_cm.__enter__()

        # resident weights: [128, KD|KH, cols] tile-major layouts
        w1_sb = wp.tile([P, KD, H], f32r)
        nc.sync.dma_start(w1_sb[:], w1.ap().rearrange("(kd p) h -> p kd h", p=P))
        w2_sb = wp.tile([P, KH, D], f32r)
        nc.sync.dma_start(w2_sb[:], w2.ap().rearrange("(kh p) d -> p kh d", p=P))
        v1_sb = wp.tile([P, KD, H], f32r)
        nc.sync.dma_start(v1_sb[:], v1.ap().rearrange("(kd p) h -> p kd h", p=P))
        v2_sb = wp.tile([P, KH, D], f32r)
        nc.sync.dma_start(v2_sb[:], v2.ap().rearrange("(kh p) d -> p kh d", p=P))
        b1_sb = wp.tile([P, KH], f32)
        nc.sync.dma_start(b1_sb[:], b1.ap())
        c1_sb = wp.tile([P, KH], f32)
        nc.sync.dma_start(c1_sb[:], c1.ap())
        b2_sb = wp.tile([1, D], f32r)
        nc.sync.dma_start(b2_sb[:], b2.ap())
        c2_sb = wp.tile([1, D], f32r)
        nc.sync.dma_start(c2_sb[:], c2.ap())
        sw_sb = wp.tile([1, C], f32r)
        nc.sync.dma_start(sw_sb[:], sw.ap())
        on_sb = wp.tile([1, 512], f32r)
        nc.sync.dma_start(on_sb[:], on.ap())

        c0 = 0
        for N in chunks:
            csl = slice(c0, c0 + N)
            xt_t = xp.tile([P, KD, N], f32r, tag="xt")
            nc.sync.dma_start(
                xt_t[:], xt.ap()[:, csl].rearrange("(kd p) n -> p kd n", p=P))
            # per-token shared scale broadcast to all 128 partitions
            swf = sp.tile([P, N], f32r, tag="swf")
            nc.sync.dma_start(
                swf[:],
                bass.AP(tensor=sw.ap().tensor, offset=c0, ap=[[0, P], [1, N]]))

            l2ps = []
            for dt in range(ND):
                l2ps.append(l2p.tile([P, N], f32, tag="l2", name=f"l2_{dt}"))

            # ---- phase A: expert path.  L1 into h tiles, L2 accumulates. ----
            # ---- phase B: shared path.  Same, h tiles scaled by swf.      ----
            for phase in range(2):
                m1_sb = w1_sb if phase == 0 else v1_sb
                m2_sb = w2_sb if phase == 0 else v2_sb
                bias = b1_sb if phase == 0 else c1_sb
                h_tiles = [None] * KH

                def emit_l1(kh):
                    ps = l1p.tile([P, N], f32, tag="l1")
                    for kd in range(KD):
                        nc.tensor.matmul(
                            ps[:], m1_sb[:, kd, kh * P:(kh + 1) * P],
                            xt_t[:, kd, :], start=(kd == 0), stop=(kd == KD - 1))
                    ht = hp.tile([P, N], f32r, tag="h")
                    nc.scalar.activation(ht[:], ps[:], Gelu,
                                         bias=bias[:, kh:kh + 1])
                    if phase == 1:
                        hs = hp.tile([P, N], f32r, tag="hs")
                        nc.vector.tensor_mul(hs[:], ht[:], swf[:])
                        ht = hs
                    h_tiles[kh] = ht

                def emit_l2(kh):
                    ht = h_tiles[kh]
                    first = (phase == 0 and kh == 0)
                    for dt in range(ND):
                        nc.tensor.matmul(
                            l2ps[dt][:], m2_sb[:, kh, dt * P:(dt + 1) * P],
                            ht[:], start=first, stop=False)

                LOOKAHEAD = 2
                for kh in range(KH + LOOKAHEAD):
                    if kh < KH:
                        emit_l1(kh)
                    if kh >= LOOKAHEAD:
                        emit_l2(kh - LOOKAHEAD)

            # rank-1 bias terms: + b2 x ones + sb2 x sw, then close the group
            for dt in range(ND):
                dsl = slice(dt * P, (dt + 1) * P)
                nc.tensor.matmul(l2ps[dt][:], b2_sb[0:1, dsl], on_sb[0:1, :N],
                                 start=False, stop=False)
                nc.tensor.matmul(l2ps[dt][:], c2_sb[0:1, dsl], sw_sb[0:1, csl],
                                 start=False, stop=True)
                o = op.tile([P, N], f32, tag="o")
                nc.vector.tensor_copy(o[:], l2ps[dt][:])
                nc.sync.dma_start(out.ap()[dsl, csl], o[:])
            c0 += N
        if repeat > 1:
            loop_cm.__exit__(None, None, None)
    nc.compile()
    return nc


def _get_program(chunks, repeat=1):
    key = (chunks, repeat)
    if key not in _prog_cache:
        _prog_cache[key] = _build_program(chunks, repeat)
    return _prog_cache[key]


def kernel(x, gW, gb, W1, b1, W2, b2, sW1, sb1, sW2, sb2, shared_weight):
    from concourse.bass_utils import run_bass_kernel_spmd

    x = np.asarray(x, dtype=np.float32)
    gW = np.asarray(gW, dtype=np.float32)
    gb = np.asarray(gb, dtype=np.float32)
    W1 = np.asarray(W1, dtype=np.float32)
    b1 = np.asarray(b1, dtype=np.float32)
    W2 = np.asarray(W2, dtype=np.float32)
    b2 = np.asarray(b2, dtype=np.float32)
    sW1 = np.asarray(sW1, dtype=np.float32)
    sb1 = np.asarray(sb1, dtype=np.float32)
    sW2 = np.asarray(sW2, dtype=np.float32)
    sb2 = np.asarray(sb2, dtype=np.float32)
    shared_weight = np.float32(np.asarray(shared_weight))

    B, S, d = x.shape
    T = B * S
    xf = x.reshape(T, d)

    # ---- host router (part of the dispatch/sharding step) ----
    # float64 logits give a numerically stable argmax for dispatch; the
    # returned gate is computed in f32 like the reference.
    logits64 = xf.astype(np.float64) @ gW.astype(np.float64) + gb
    logits = logits64.astype(np.float32)
    m = logits.max(axis=-1, keepdims=True)
    eg = np.exp(logits - m)
    gate = eg / eg.sum(axis=-1, keepdims=True)          # [T, E] f32
    idx = logits64.argmax(axis=-1)                      # [T]
    conf = gate.max(axis=-1)                            # [T]
    swt = ((1.0 - conf) * (1.0 / (1.0 + np.exp(-float(shared_weight))))
           ).astype(np.float32)                         # [T]

    counts = np.bincount(idx, minlength=E)
    aux = float((counts / T * gate.mean(axis=0, dtype=np.float64)).sum() * E * LBW)

    # ---- bin-pack (expert, token-slice) items onto 8 cores ----
    chunks = CHUNKS_PRIMARY
    C = sum(chunks)
    if sum(-(-int(n) // C) for n in counts) > NCORES:
        chunks = CHUNKS_FALLBACK
        C = sum(chunks)

    order = np.argsort(idx, kind="stable")
    starts = np.zeros(E + 1, dtype=np.int64)
    starts[1:] = np.cumsum(counts)
    items = []  # (expert, token_index_array)
    for e in range(E):
        toks = order[starts[e]:starts[e + 1]]
        nslots = max(1, -(-len(toks) // C))
        for part in np.array_split(toks, nslots):
            items.append((e, part))
    while len(items) < NCORES:
        items.append((0, np.empty(0, dtype=np.int64)))
    assert len(items) == NCORES, f"bin-packing failed: {len(items)} items"

    nc = _get_program(chunks)

    ones = np.ones((1, 512), dtype=np.float32)
    zcol = np.zeros((P, KH), dtype=np.float32)
    sb1_r = np.ascontiguousarray(sb1.reshape(KH, P).T)
    in_maps = []
    for e, toks in items:
        n = len(toks)
        xt_c = np.zeros((D, C), dtype=np.float32)
        if n:
            xt_c[:, :n] = xf[toks].T
        sw_c = np.zeros((1, C), dtype=np.float32)
        if n:
            sw_c[0, :n] = swt[toks]
        in_maps.append({
            "xt": xt_c,
            "w1": np.ascontiguousarray(W1[e]),
            "w2": np.ascontiguousarray(W2[e]),
            "v1": sW1,
            "v2": sW2,
            "b1": np.ascontiguousarray(b1[e].reshape(KH, P).T),
            "c1": sb1_r,
            "b2": np.ascontiguousarray(b2[e][None, :]),
            "c2": np.ascontiguousarray(sb2[None, :]),
            "sw": sw_c,
            "on": ones,
        })

    res = run_bass_kernel_spmd(nc, in_maps, core_ids=list(range(NCORES)))

    y = np.empty((T, D), dtype=np.float32)
    for (e, toks), r in zip(items, res.results):
        n = len(toks)
        if n:
            y[toks] = r["out"][:, :n].T

    return (y.reshape(B, S, D), np.float32(aux),
            gate.reshape(B, S, E))


# revision 6
# speedup vs baseline: 1.1832x; 1.1832x over previous
"""MoE with shared expert — Trainium2 Bass kernel, 8-core expert-parallel.

Strategy (per sharding hint): tokens are dispatched by argmax expert on the
host at input-sharding time; the 8 cores each hold ONE expert's weights plus
the replicated shared-expert weights, and run the fused two-layer MLP
(selected expert + shared expert, combined in PSUM) over their token slice.
The tiny router (T x 512 x 4 matmul + softmax) and the final gather/scatter
run on the host as part of sharding/unsharding; the device does the ~69
GFLOP of expert/shared MLP work.

Device kernel, per core, for capacity C tokens (chunks of <=512):
  hT_e  = gelu(W1e^T @ X^T + b1e)            [H, C]   (streamed per 128-row tile)
  acc   = W2e^T @ hT_e                        [D, C]   (PSUM, 4 banks)
  hT_s  = gelu(sW1^T @ X^T + sb1) * sw        [H, C]   (sw = per-token scale)
  acc  += sW2^T @ hT_s + b2e x 1 + sb2 x sw   (rank-1 bias matmuls, K=1)
All matmuls run as float32r (TF32-like, full PE rate at N>=256).
"""

import numpy as np
from contextlib import ExitStack

D, H, E = 512, 2048, 4
MM_DTYPE = "bfloat16"  # "float32r" for higher precision, "bfloat16" for speed
NCORES = 8
LBW = 0.01
P = 128
KD, KH, ND = D // P, H // P, D // P

# token capacity per core and the matmul chunk sizes that tile it
CHUNKS_PRIMARY = (512, 384, 256)     # C = 1152, covers expert counts <= 2304
CHUNKS_FALLBACK = (512, 512, 512, 512)  # C = 2048, covers any distribution

_prog_cache = {}


def _build_program(chunks, repeat=1):
    """Build the per-core Bacc program.  repeat>1 wraps the whole body
    (including weight loads) in a For_i loop — used only for timing."""
    import concourse.tile as tile
    from concourse import bacc, mybir

    f32 = mybir.dt.float32
    f32r = getattr(mybir.dt, MM_DTYPE)
    Gelu = mybir.ActivationFunctionType.Gelu
    C = sum(chunks)

    nc = bacc.Bacc("TRN2", target_bir_lowering=False, debug=False)
    xt = nc.dram_tensor("xt", [D, C], f32r, kind="ExternalInput")
    w1 = nc.dram_tensor("w1", [D, H], f32r, kind="ExternalInput")
    w2 = nc.dram_tensor("w2", [H, D], f32r, kind="ExternalInput")
    v1 = nc.dram_tensor("v1", [D, H], f32r, kind="ExternalInput")
    v2 = nc.dram_tensor("v2", [H, D], f32r, kind="ExternalInput")
    b1 = nc.dram_tensor("b1", [P, KH], f32, kind="ExternalInput")
    c1 = nc.dram_tensor("c1", [P, KH], f32, kind="ExternalInput")
    b2 = nc.dram_tensor("b2", [1, D], f32r, kind="ExternalInput")
    c2 = nc.dram_tensor("c2", [1, D], f32r, kind="ExternalInput")
    sw = nc.dram_tensor("sw", [1, C], f32r, kind="ExternalInput")
    on = nc.dram_tensor("on", [1, 512], f32r, kind="ExternalInput")
    out = nc.dram_tensor("out", [D, C], f32, kind="ExternalOutput")

    import concourse.bass as bass

    with tile.TileContext(nc) as tc, ExitStack() as ctx:
        wp = ctx.enter_context(tc.tile_pool(name="wp", bufs=1))
        xp = ctx.enter_context(tc.tile_pool(name="xp", bufs=2))
        hp = ctx.enter_context(tc.tile_pool(name="hp", bufs=4))
        sp = ctx.enter_context(tc.tile_pool(name="sp", bufs=2))
        op = ctx.enter_context(tc.tile_pool(name="op", bufs=2))
        l1p = ctx.enter_context(tc.tile_pool(name="l1p", bufs=3, space="PSUM"))
        l2p = ctx.enter_context(tc.tile_pool(name="l2p", bufs=4, space="PSUM"))

        if repeat > 1:
            loop_cm = tc.For_i(0, repeat, 1,
                               hint_engines=(mybir.EngineType.PE,))
            loop_cm.__enter__()

        # resident weights: [128, KD|KH, cols] tile-major layouts
        w1_sb = wp.tile([P, KD, H], f32r)
        nc.sync.dma_start(w1_sb[:], w1.ap().rearrange("(kd p) h -> p kd h", p=P))
        w2_sb = wp.tile([P, KH, D], f32r)
        nc.sync.dma_start(w2_sb[:], w2.ap().rearrange("(kh p) d -> p kh d", p=P))
        v1_sb = wp.tile([P, KD, H], f32r)
        nc.sync.dma_start(v1_sb[:], v1.ap().rearrange("(kd p) h -> p kd h", p=P))
        v2_sb = wp.tile([P, KH, D], f32r)
        nc.sync.dma_start(v2_sb[:], v2.ap().rearrange("(kh p) d -> p kh d", p=P))
        b1_sb = wp.tile([P, KH], f32)
        nc.sync.dma_start(b1_sb[:], b1.ap())
        c1_sb = wp.tile([P, KH], f32)
        nc.sync.dma_start(c1_sb[:], c1.ap())
        b2_sb = wp.tile([1, D], f32r)
        nc.sync.dma_start(b2_sb[:], b2.ap())
        c2_sb = wp.tile([1, D], f32r)
        nc.sync.dma_start(c2_sb[:], c2.ap())
        sw_sb = wp.tile([1, C], f32r)
        nc.sync.dma_start(sw_sb[:], sw.ap())
        on_sb = wp.tile([1, 512], f32r)
        nc.sync.dma_start(on_sb[:], on.ap())

        c0 = 0
        for N in chunks:
            csl = slice(c0, c0 + N)
            xt_t = xp.tile([P, KD, N], f32r, tag="xt")
            nc.sync.dma_start(
                xt_t[:], xt.ap()[:, csl].rearrange("(kd p) n -> p kd n", p=P))
            # per-token shared scale broadcast to all 128 partitions
            swf = sp.tile([P, N], f32r, tag="swf")
            nc.sync.dma_start(
                swf[:],
                bass.AP(tensor=sw.ap().tensor, offset=c0, ap=[[0, P], [1, N]]))

            l2ps = []
            for dt in range(ND):
                l2ps.append(l2p.tile([P, N], f32, tag="l2", name=f"l2_{dt}"))

            # ---- phase A: expert path.  L1 into h tiles, L2 accumulates. ----
            # ---- phase B: shared path.  Same, h tiles scaled by swf.      ----
            for phase in range(2):
                m1_sb = w1_sb if phase == 0 else v1_sb
                m2_sb = w2_sb if phase == 0 else v2_sb
                bias = b1_sb if phase == 0 else c1_sb
                h_tiles = [None] * KH

                def emit_l1(kh):
                    ps = l1p.tile([P, N], f32, tag="l1")
                    for kd in range(KD):
                        nc.tensor.matmul(
                            ps[:], m1_sb[:, kd, kh * P:(kh + 1) * P],
                            xt_t[:, kd, :], start=(kd == 0), stop=(kd == KD - 1))
                    ht = hp.tile([P, N], f32r, tag="h")
                    nc.scalar.activation(ht[:], ps[:], Gelu,
                                         bias=bias[:, kh:kh + 1])
                    if phase == 1:
                        hs = hp.tile([P, N], f32r, tag="hs")
                        nc.vector.tensor_mul(hs[:], ht[:], swf[:])
                        ht = hs
                    h_tiles[kh] = ht

                def emit_l2(kh):
                    ht = h_tiles[kh]
                    first = (phase == 0 and kh == 0)
                    for dt in range(ND):
                        nc.tensor.matmul(
                            l2ps[dt][:], m2_sb[:, kh, dt * P:(dt + 1) * P],
                            ht[:], start=first, stop=False)

                LOOKAHEAD = 2
                for kh in range(KH + LOOKAHEAD):
                    if kh < KH:
                        emit_l1(kh)
                    if kh >= LOOKAHEAD:
                        emit_l2(kh - LOOKAHEAD)

            # rank-1 bias terms: + b2 x ones + sb2 x sw, then close the group
            for dt in range(ND):
                dsl = slice(dt * P, (dt + 1) * P)
                nc.tensor.matmul(l2ps[dt][:], b2_sb[0:1, dsl], on_sb[0:1, :N],
                                 start=False, stop=False)
                nc.tensor.matmul(l2ps[dt][:], c2_sb[0:1, dsl], sw_sb[0:1, csl],
                                 start=False, stop=True)
                o = op.tile([P, N], f32, tag="o")
                nc.vector.tensor_copy(o[:], l2ps[dt][:])
                nc.sync.dma_start(out.ap()[dsl, csl], o[:])
            c0 += N
        if repeat > 1:
            loop_cm.__exit__(None, None, None)
    nc.compile()
    return nc


def _get_program(chunks, repeat=1):
    key = (chunks, repeat)
    if key not in _prog_cache:
        _prog_cache[key] = _build_program(chunks, repeat)
    return _prog_cache[key]


def kernel(x, gW, gb, W1, b1, W2, b2, sW1, sb1, sW2, sb2, shared_weight):
    from concourse.bass_utils import run_bass_kernel_spmd

    x = np.asarray(x, dtype=np.float32)
    gW = np.asarray(gW, dtype=np.float32)
    gb = np.asarray(gb, dtype=np.float32)
    W1 = np.asarray(W1, dtype=np.float32)
    b1 = np.asarray(b1, dtype=np.float32)
    W2 = np.asarray(W2, dtype=np.float32)
    b2 = np.asarray(b2, dtype=np.float32)
    sW1 = np.asarray(sW1, dtype=np.float32)
    sb1 = np.asarray(sb1, dtype=np.float32)
    sW2 = np.asarray(sW2, dtype=np.float32)
    sb2 = np.asarray(sb2, dtype=np.float32)
    shared_weight = np.float32(np.asarray(shared_weight))

    B, S, d = x.shape
    T = B * S
    xf = x.reshape(T, d)

    # ---- host router (part of the dispatch/sharding step) ----
    # float64 logits give a numerically stable argmax for dispatch; the
    # returned gate is computed in f32 like the reference.
    logits64 = xf.astype(np.float64) @ gW.astype(np.float64) + gb
    logits = logits64.astype(np.float32)
    m = logits.max(axis=-1, keepdims=True)
    eg = np.exp(logits - m)
    gate = eg / eg.sum(axis=-1, keepdims=True)          # [T, E] f32
    idx = logits64.argmax(axis=-1)                      # [T]
    conf = gate.max(axis=-1)                            # [T]
    swt = ((1.0 - conf) * (1.0 / (1.0 + np.exp(-float(shared_weight))))
           ).astype(np.float32)                         # [T]

    counts = np.bincount(idx, minlength=E)
    aux = float((counts / T * gate.mean(axis=0, dtype=np.float64)).sum() * E * LBW)

    # ---- bin-pack (expert, token-slice) items onto 8 cores ----
    chunks = CHUNKS_PRIMARY
    C = sum(chunks)
    if sum(-(-int(n) // C) for n in counts) > NCORES:
        chunks = CHUNKS_FALLBACK
        C = sum(chunks)

    order = np.argsort(idx, kind="stable")
    starts = np.zeros(E + 1, dtype=np.int64)
    starts[1:] = np.cumsum(counts)
    items = []  # (expert, token_index_array)
    for e in range(E):
        toks = order[starts[e]:starts[e + 1]]
        nslots = max(1, -(-len(toks) // C))
        for part in np.array_split(toks, nslots):
            items.append((e, part))
    while len(items) < NCORES:
        items.append((0, np.empty(0, dtype=np.int64)))
    assert len(items) == NCORES, f"bin-packing failed: {len(items)} items"

    nc = _get_program(chunks)

    import ml_dtypes
    mdt = np.float32 if MM_DTYPE == "float32r" else ml_dtypes.bfloat16
    ones = np.ones((1, 512), dtype=mdt)
    sb1_r = np.ascontiguousarray(sb1.reshape(KH, P).T)
    xf_m = xf.astype(mdt)
    sW1_m = sW1.astype(mdt)
    sW2_m = sW2.astype(mdt)
    sb2_m = np.ascontiguousarray(sb2[None, :]).astype(mdt)
    in_maps = []
    for e, toks in items:
        n = len(toks)
        xt_c = np.zeros((D, C), dtype=mdt)
        if n:
            xt_c[:, :n] = xf_m[toks].T
        sw_c = np.zeros((1, C), dtype=np.float32)
        if n:
            sw_c[0, :n] = swt[toks]
        in_maps.append({
            "xt": xt_c,
            "w1": np.ascontiguousarray(W1[e]).astype(mdt),
            "w2": np.ascontiguousarray(W2[e]).astype(mdt),
            "v1": sW1_m,
            "v2": sW2_m,
            "b1": np.ascontiguousarray(b1[e].reshape(KH, P).T),
            "c1": sb1_r,
            "b2": np.ascontiguousarray(b2[e][None, :]).astype(mdt),
            "c2": sb2_m,
            "sw": sw_c.astype(mdt),
            "on": ones,
        })

    res = run_bass_kernel_spmd(nc, in_maps, core_ids=list(range(NCORES)))

    y = np.empty((T, D), dtype=np.float32)
    for (e, toks), r in zip(items, res.results):
        n = len(toks)
        if n:
            y[toks] = r["out"][:, :n].T

    return (y.reshape(B, S, D), np.float32(aux),
            gate.reshape(B, S, E))


# revision 17
# speedup vs baseline: 1.3207x; 1.1162x over previous
"""MoE with shared expert — Trainium2 Bass kernel, 8-core expert-parallel.

Strategy (per sharding hint): tokens are dispatched by argmax expert on the
host at input-sharding time; the 8 cores each hold ONE expert's weights plus
the replicated shared-expert weights, and run the fused two-layer MLP
(selected expert + shared expert, combined in PSUM) over their token slice.
The tiny router (T x 512 x 4 matmul + softmax) and the final gather/scatter
run on the host as part of sharding/unsharding; the device does the ~69
GFLOP of expert/shared MLP work.

Device kernel, per core, for capacity C tokens (chunks of <=512):
  hT_e  = gelu(W1e^T @ X^T + b1e)            [H, C]   (streamed per 128-row tile)
  acc   = W2e^T @ hT_e                        [D, C]   (PSUM, 4 banks)
  hT_s  = gelu(sW1^T @ X^T + sb1) * sw        [H, C]   (sw = per-token scale)
  acc  += sW2^T @ hT_s + b2e x 1 + sb2 x sw   (rank-1 bias matmuls, K=1)
All matmuls run as float32r (TF32-like, full PE rate at N>=256).
"""

import numpy as np
from contextlib import ExitStack

D, H, E = 512, 2048, 4
MM_DTYPE = "bfloat16"  # "float32r" for higher precision, "bfloat16" for speed
NCORES = 8
LBW = 0.01
P = 128
KD, KH, ND = D // P, H // P, D // P

# token capacity per core and the matmul chunk sizes that tile it
CHUNKS_PRIMARY = (512, 384, 256)     # C = 1152, covers expert counts <= 2304
CHUNKS_FALLBACK = (512, 512, 512, 512)  # C = 2048, covers any distribution

_prog_cache = {}


def plan_slots(counts):
    """Assign the 8 cores to experts (LPT greedy): expert e gets s_e cores,
    sum(s_e) == NCORES, minimising the max per-core token count."""
    counts = [int(n) for n in counts]
    slots = [1] * len(counts)
    for _ in range(NCORES - len(counts)):
        i = max(range(len(counts)), key=lambda j: -(-counts[j] // slots[j]))
        slots[i] += 1
    C = max(-(-n // s) for n, s in zip(counts, slots))
    return slots, max(C, 64)


def plan_chunks(C):
    """Split capacity C into k equal chunks of <= 512, multiples of 8
    (bf16 matmuls run at ~N cycles for any N, so any chunk size works)."""
    k = -(-C // 512)
    C = -(-C // (8 * k)) * (8 * k)
    return tuple([C // k] * k)


def _build_program(chunks, repeat=1):
    """Build the per-core Bacc program.  repeat>1 wraps the whole body
    (including weight loads) in a For_i loop — used only for timing."""
    import concourse.tile as tile
    from concourse import bacc, mybir

    f32 = mybir.dt.float32
    f32r = getattr(mybir.dt, MM_DTYPE)
    Gelu = mybir.ActivationFunctionType.Gelu
    C = sum(chunks)

    nc = bacc.Bacc("TRN2", target_bir_lowering=False, debug=False)
    xt = nc.dram_tensor("xt", [D, C], f32r, kind="ExternalInput")
    w1 = nc.dram_tensor("w1", [D, H], f32r, kind="ExternalInput")
    w2 = nc.dram_tensor("w2", [H, D], f32r, kind="ExternalInput")
    v1 = nc.dram_tensor("v1", [D, H], f32r, kind="ExternalInput")
    v2 = nc.dram_tensor("v2", [H, D], f32r, kind="ExternalInput")
    # b1 | sb1 packed:  [128, 2*KH] f32
    bb = nc.dram_tensor("bb", [P, 2 * KH], f32, kind="ExternalInput")
    # b2 | sb2 | sw | ones packed row:  [1, D + D + C + 512]
    mrow = nc.dram_tensor("mrow", [1, 2 * D + C + 512], f32r,
                          kind="ExternalInput")
    out = nc.dram_tensor("out", [D, C], f32, kind="ExternalOutput")

    import concourse.bass as bass

    with tile.TileContext(nc) as tc, ExitStack() as ctx:
        wp = ctx.enter_context(tc.tile_pool(name="wp", bufs=1))
        xp = ctx.enter_context(tc.tile_pool(name="xp", bufs=2))
        hp = ctx.enter_context(tc.tile_pool(name="hp", bufs=4))
        sp = ctx.enter_context(tc.tile_pool(name="sp", bufs=2))
        op = ctx.enter_context(tc.tile_pool(name="op", bufs=2))
        l1p = ctx.enter_context(tc.tile_pool(name="l1p", bufs=3, space="PSUM"))
        l2p = ctx.enter_context(tc.tile_pool(name="l2p", bufs=4, space="PSUM"))

        if repeat > 1:
            loop_cm = tc.For_i(0, repeat, 1,
                               hint_engines=(mybir.EngineType.PE,))
            loop_cm.__enter__()

        offs = [sum(chunks[:i]) for i in range(len(chunks))]
        chunk_inputs = {}

        def emit_chunk_inputs(ci):
            c0, N = offs[ci], chunks[ci]
            xt_t = xp.tile([P, KD, N], f32r, tag="xt", name=f"xt_{ci}")
            nc.sync.dma_start(
                xt_t[:],
                xt.ap()[:, c0:c0 + N].rearrange("(kd p) n -> p kd n", p=P))
            # per-token shared scale broadcast to all 128 partitions
            swf = sp.tile([P, N], f32r, tag="swf", name=f"swf_{ci}")
            nc.sync.dma_start(
                swf[:],
                bass.AP(tensor=mrow.ap().tensor, offset=2 * D + c0,
                        ap=[[0, P], [1, N]]))
            chunk_inputs[ci] = (xt_t, swf)

        # chunk-0 inputs first so the PE can start as soon as w1's first
        # block lands; weights follow, w1 split so the wait is short.
        emit_chunk_inputs(0)
        w1_sb = wp.tile([P, KD, H], f32r)
        w1_src = w1.ap().rearrange("(kd p) h -> p kd h", p=P)
        nc.sync.dma_start(w1_sb[:, :, 0:H // 4], w1_src[:, :, 0:H // 4])
        bb_sb = wp.tile([P, 2 * KH], f32)
        nc.sync.dma_start(bb_sb[:], bb.ap())
        mrow_sb = wp.tile([1, 2 * D + C + 512], f32r)
        nc.sync.dma_start(mrow_sb[:], mrow.ap())
        b1_sb = bb_sb[:, 0:KH]
        c1_sb = bb_sb[:, KH:2 * KH]
        b2_sb = mrow_sb[:, 0:D]
        c2_sb = mrow_sb[:, D:2 * D]
        sw_sb = mrow_sb[:, 2 * D:2 * D + C]
        on_sb = mrow_sb[:, 2 * D + C:]
        for blk in range(1, 4):
            hsl = slice(blk * (H // 4), (blk + 1) * (H // 4))
            nc.sync.dma_start(w1_sb[:, :, hsl], w1_src[:, :, hsl])
        w2_sb = wp.tile([P, KH, D], f32r)
        w2_src = w2.ap().rearrange("(kh p) d -> p kh d", p=P)
        for blk in range(2):
            ksl = slice(blk * (KH // 2), (blk + 1) * (KH // 2))
            nc.sync.dma_start(w2_sb[:, ksl, :], w2_src[:, ksl, :])
        v1_sb = wp.tile([P, KD, H], f32r)
        nc.sync.dma_start(v1_sb[:], v1.ap().rearrange("(kd p) h -> p kd h", p=P))
        v2_sb = wp.tile([P, KH, D], f32r)
        nc.sync.dma_start(v2_sb[:], v2.ap().rearrange("(kh p) d -> p kh d", p=P))

        for ci, N in enumerate(chunks):
            c0 = offs[ci]
            csl = slice(c0, c0 + N)
            xt_t, swf = chunk_inputs[ci]

            l2ps = []
            for dt in range(ND):
                l2ps.append(l2p.tile([P, N], f32, tag="l2", name=f"l2_{dt}"))

            # ---- phase A: expert path, streaming h tiles, L2 one behind ----
            h_tiles = [None] * KH

            def emit_l1a(kh):
                ps = l1p.tile([P, N], f32, tag="l1", name="l1a")
                for kd in range(KD):
                    nc.tensor.matmul(
                        ps[:], w1_sb[:, kd, kh * P:(kh + 1) * P],
                        xt_t[:, kd, :], start=(kd == 0), stop=(kd == KD - 1))
                ht = hp.tile([P, N], f32r, tag="h", name="ha")
                nc.scalar.activation(ht[:], ps[:], Gelu, bias=b1_sb[:, kh:kh + 1])
                h_tiles[kh] = ht

            def emit_l2a(kh):
                for dt in range(ND):
                    nc.tensor.matmul(
                        l2ps[dt][:], w2_sb[:, kh, dt * P:(dt + 1) * P],
                        h_tiles[kh][:], start=(kh == 0), stop=False)

            LOOKAHEAD = 2
            for kh in range(KH + LOOKAHEAD):
                if kh < KH:
                    emit_l1a(kh)
                if kh >= LOOKAHEAD:
                    emit_l2a(kh - LOOKAHEAD)

            # ---- phase B: shared path.  L1 into hs_all (scaled by swf),
            # then L2 dt-major so each output tile closes + drains early. ----
            hs_all = hp.tile([P, KH, N], f32r, tag="hsall", name="hsall", bufs=2)
            for kh in range(KH):
                ps = l1p.tile([P, N], f32, tag="l1", name="l1b")
                for kd in range(KD):
                    nc.tensor.matmul(
                        ps[:], v1_sb[:, kd, kh * P:(kh + 1) * P],
                        xt_t[:, kd, :], start=(kd == 0), stop=(kd == KD - 1))
                ht = hp.tile([P, N], f32r, tag="h", name="hb")
                nc.scalar.activation(ht[:], ps[:], Gelu, bias=c1_sb[:, kh:kh + 1])
                nc.vector.tensor_mul(hs_all[:, kh, :], ht[:], swf[:])

            # prefetch next chunk's inputs before this chunk's output DMAs
            if ci + 1 < len(chunks):
                emit_chunk_inputs(ci + 1)

            for dt in range(ND):
                dsl = slice(dt * P, (dt + 1) * P)
                for kh in range(KH):
                    nc.tensor.matmul(
                        l2ps[dt][:], v2_sb[:, kh, dt * P:(dt + 1) * P],
                        hs_all[:, kh, :], start=False, stop=False)
                nc.tensor.matmul(l2ps[dt][:], b2_sb[0:1, dsl], on_sb[0:1, :N],
                                 start=False, stop=False)
                nc.tensor.matmul(l2ps[dt][:], c2_sb[0:1, dsl], sw_sb[0:1, csl],
                                 start=False, stop=True)
                o = op.tile([P, N], f32, tag="o", name="o")
                nc.vector.tensor_copy(o[:], l2ps[dt][:])
                nc.sync.dma_start(out.ap()[dsl, csl], o[:])
        if repeat > 1:
            loop_cm.__exit__(None, None, None)
    nc.compile()
    return nc


def _get_program(chunks, repeat=1):
    key = (chunks, repeat)
    if key not in _prog_cache:
        _prog_cache[key] = _build_program(chunks, repeat)
    return _prog_cache[key]


def prepare(x, gW, gb, W1, b1, W2, b2, sW1, sb1, sW2, sb2, shared_weight):
    """Host-side routing + sharding.  Returns (chunks, in_maps, items, gate,
    aux) where items[i] = (expert, token_indices) for core i."""
    x = np.asarray(x, dtype=np.float32)
    gW = np.asarray(gW, dtype=np.float32)
    gb = np.asarray(gb, dtype=np.float32)
    W1 = np.asarray(W1, dtype=np.float32)
    b1 = np.asarray(b1, dtype=np.float32)
    W2 = np.asarray(W2, dtype=np.float32)
    b2 = np.asarray(b2, dtype=np.float32)
    sW1 = np.asarray(sW1, dtype=np.float32)
    sb1 = np.asarray(sb1, dtype=np.float32)
    sW2 = np.asarray(sW2, dtype=np.float32)
    sb2 = np.asarray(sb2, dtype=np.float32)
    shared_weight = np.float32(np.asarray(shared_weight))

    B, S, d = x.shape
    T = B * S
    xf = x.reshape(T, d)

    # ---- host router (part of the dispatch/sharding step) ----
    # float64 logits give a numerically stable argmax for dispatch; the
    # returned gate is computed in f32 like the reference.
    logits64 = xf.astype(np.float64) @ gW.astype(np.float64) + gb
    logits = logits64.astype(np.float32)
    m = logits.max(axis=-1, keepdims=True)
    eg = np.exp(logits - m)
    gate = eg / eg.sum(axis=-1, keepdims=True)          # [T, E] f32
    idx = logits64.argmax(axis=-1)                      # [T]
    conf = gate.max(axis=-1)                            # [T]
    swt = ((1.0 - conf) * (1.0 / (1.0 + np.exp(-float(shared_weight))))
           ).astype(np.float32)                         # [T]

    counts = np.bincount(idx, minlength=E)
    aux = float((counts / T * gate.mean(axis=0, dtype=np.float64)).sum() * E * LBW)

    # ---- assign cores to experts, split each expert's tokens evenly ----
    slots, C = plan_slots(counts)
    chunks = plan_chunks(C)
    C = sum(chunks)

    order = np.argsort(idx, kind="stable")
    starts = np.zeros(E + 1, dtype=np.int64)
    starts[1:] = np.cumsum(counts)
    items = []  # (expert, token_index_array)
    for e in range(E):
        toks = order[starts[e]:starts[e + 1]]
        for part in np.array_split(toks, slots[e]):
            items.append((e, part))
    assert len(items) == NCORES and max(len(t) for _, t in items) <= C

    import ml_dtypes
    mdt = np.float32 if MM_DTYPE == "float32r" else ml_dtypes.bfloat16
    sb1_r = sb1.reshape(KH, P).T
    xf_m = xf.astype(mdt)
    sW1_m = sW1.astype(mdt)
    sW2_m = sW2.astype(mdt)
    in_maps = []
    for e, toks in items:
        n = len(toks)
        xt_c = np.zeros((D, C), dtype=mdt)
        if n:
            xt_c[:, :n] = xf_m[toks].T
        bb_c = np.concatenate(
            [b1[e].reshape(KH, P).T, sb1_r], axis=1)  # [P, 2*KH] f32
        mrow_c = np.zeros((1, 2 * D + C + 512), dtype=np.float32)
        mrow_c[0, 0:D] = b2[e]
        mrow_c[0, D:2 * D] = sb2
        if n:
            mrow_c[0, 2 * D:2 * D + n] = swt[toks]
        mrow_c[0, 2 * D + C:] = 1.0
        in_maps.append({
            "xt": xt_c,
            "w1": np.ascontiguousarray(W1[e]).astype(mdt),
            "w2": np.ascontiguousarray(W2[e]).astype(mdt),
            "v1": sW1_m,
            "v2": sW2_m,
            "bb": np.ascontiguousarray(bb_c),
            "mrow": mrow_c.astype(mdt),
        })
    return chunks, in_maps, items, gate, aux


def kernel(x, gW, gb, W1, b1, W2, b2, sW1, sb1, sW2, sb2, shared_weight):
    from concourse.bass_utils import run_bass_kernel_spmd

    x = np.asarray(x, dtype=np.float32)
    B, S, _ = x.shape
    T = B * S
    chunks, in_maps, items, gate, aux = prepare(
        x, gW, gb, W1, b1, W2, b2, sW1, sb1, sW2, sb2, shared_weight)
    nc = _get_program(chunks)
    res = run_bass_kernel_spmd(nc, in_maps, core_ids=list(range(NCORES)))

    y = np.empty((T, D), dtype=np.float32)
    for (e, toks), r in zip(items, res.results):
        n = len(toks)
        if n:
            y[toks] = r["out"][:, :n].T

    return (y.reshape(B, S, D), np.float32(aux),
            gate.reshape(B, S, E))
